# revision 15
# baseline (speedup 1.0000x reference)
"""2-layer GCN (PyG GCNConv semantics) on 8 Trainium2 NeuronCores.

Math: out = A_n @ relu(A_n @ x @ W1 + b1) @ W2 + b2, where A_n is the
self-loop-augmented, symmetrically normalized adjacency.

Key reordering: A_n @ (x @ W1) == (A_n @ x) @ W1, so we propagate the
128-dim x (not the 500-dim hidden) in layer 1.  Layer 2 propagates the
scalar h2 = relu(H1) @ W2.

Sharding: nodes are split into 8 contiguous shards of 6272 (49 blocks of
128 dsts).  Edges (incl. self-loops) are partitioned by dst shard.  Per
128-dst block, edges are packed into [128-lane x T_b-tile] arrays; a
weighted one-hot matrix S_w[e, d] = (dst_e == d) * w_e is built per tile
with one dual-op tensor_scalar, and TensorE matmuls against gathered
src rows perform the segment-sum in PSUM.  Layer-2 scalar features are
AllGathered so pass 2 reuses the identical block structure.
"""

import numpy as np

P = 128
D_FEAT = 128
D_HID = 500
HCH = 125  # hidden chunk (D_HID = HCH * NCH)
NCH = 4
N_CORES = 8


def _preprocess(edge_index, n_nodes):
    bpc = int(np.ceil(n_nodes / (N_CORES * P)))  # blocks per core
    shard = bpc * P
    src = np.asarray(edge_index[0], dtype=np.int64)
    dst = np.asarray(edge_index[1], dtype=np.int64)
    loops = np.arange(n_nodes, dtype=np.int64)
    s = np.concatenate([src, loops])
    d = np.concatenate([dst, loops])
    deg = np.bincount(d, minlength=n_nodes).astype(np.float32)
    dinv = (1.0 / np.sqrt(deg)).astype(np.float32)
    w = dinv[s] * dinv[d]

    gblk = d // P  # global 128-dst block id; (core, blk) = divmod(gblk, bpc)
    order = np.argsort(gblk, kind="stable")
    s, d, w, gblk = s[order], d[order], w[order], gblk[order]

    n_gblk = N_CORES * bpc
    counts = np.bincount(gblk, minlength=n_gblk)
    T_b = max(1, int(np.ceil(counts.max() / P)))  # tiles per block (padded)
    K = bpc * T_b  # tiles per core

    idx = np.zeros((N_CORES, P, K), dtype=np.int32)
    dstl = np.full((N_CORES, P, K), -1.0, dtype=np.float32)
    wv = np.zeros((N_CORES, P, K), dtype=np.float32)

    starts = np.zeros(n_gblk + 1, dtype=np.int64)
    np.cumsum(counts, out=starts[1:])
    r = np.arange(len(d), dtype=np.int64) - starts[gblk]
    core_e = gblk // bpc
    b_e = gblk % bpc
    tile_e = b_e * T_b + r // P
    lane_e = r % P
    idx[core_e, lane_e, tile_e] = s.astype(np.int32)
    dstl[core_e, lane_e, tile_e] = (d % P).astype(np.float32)
    wv[core_e, lane_e, tile_e] = w
    return idx, dstl, wv, bpc, T_b, K


def _build_program(n_nodes, K, T_b, bpc, b2_val):
    from concourse import bass, mybir
    from concourse.bacc import Bacc
    import concourse.tile as tile

    f32 = mybir.dt.float32
    i32 = mybir.dt.int32
    shard = bpc * P
    pad_n = N_CORES * shard

    nc = Bacc()
    # meta packs [dstl | wv | iota] so one DMA (one semaphore source)
    # covers every DVE constant; three separate loads overflowed the
    # tensor_scalar instruction's sync-wait slots at codegen.
    MW = 2 * K + P
    WW = D_HID + 2 * NCH  # [W1 | b1 cols | w2 cols]
    x_d = nc.dram_tensor("x", [n_nodes, D_FEAT], f32, kind="ExternalInput")
    idx_d = nc.dram_tensor("idx", [P, K], i32, kind="ExternalInput")
    meta_d = nc.dram_tensor("meta", [P, MW], f32, kind="ExternalInput")
    wts_d = nc.dram_tensor("wts", [P, WW], f32, kind="ExternalInput")
    out_d = nc.dram_tensor("out", [shard, 1], f32, kind="ExternalOutput")

    eq = mybir.AluOpType.is_equal
    mul = mybir.AluOpType.mult
    add = mybir.AluOpType.add
    mx = mybir.AluOpType.max

    with tile.TileContext(nc) as tc:
        with (
            tc.tile_pool(name="consts", bufs=1) as cpool,
            tc.tile_pool(name="gbuf", bufs=8) as gpool,
            tc.tile_pool(name="sbuf_s", bufs=4) as spool,
            tc.tile_pool(name="epi", bufs=2) as epool,
            tc.tile_pool(name="ps_acc", bufs=2, space="PSUM") as ps_acc,
            tc.tile_pool(name="ps_h1", bufs=2, space="PSUM") as ps_h1,
            tc.tile_pool(name="ps_h2", bufs=2, space="PSUM") as ps_h2,
            tc.tile_pool(name="dram", bufs=1, space="DRAM") as dpool,
        ):
            idx_t = cpool.tile([P, K], i32, name="idx_t")
            nc.sync.dma_start(out=idx_t[:], in_=idx_d[:])
            meta_t = cpool.tile([P, MW], f32, name="meta_t")
            nc.sync.dma_start(out=meta_t[:], in_=meta_d[:])
            wts_t = cpool.tile([P, WW], f32, name="wts_t")
            nc.sync.dma_start(out=wts_t[:], in_=wts_d[:])
            iota_t = meta_t[:, 2 * K : 2 * K + P]
            h2sb = cpool.tile([1, shard], f32, name="h2sb")

            # ---------------- pass 1: xa = (A_n @ x) per 128-dst block,
            # then h2 = relu(xa @ W1 + b1) @ W2 ----------------
            for b in range(bpc):
                xa_ps = ps_acc.tile([P, P], f32, name="xa_ps")
                for t in range(T_b):
                    col = b * T_b + t
                    g_t = gpool.tile([P, D_FEAT], f32, name="g_t")
                    nc.gpsimd.indirect_dma_start(
                        out=g_t[:],
                        out_offset=None,
                        in_=x_d[:],
                        in_offset=bass.IndirectOffsetOnAxis(
                            ap=idx_t[:, col : col + 1], axis=0
                        ),
                    )
                    s_t = spool.tile([P, P], f32, name="s_t")
                    nc.vector.tensor_scalar(
                        out=s_t[:],
                        in0=iota_t,
                        scalar1=meta_t[:, col : col + 1],
                        scalar2=meta_t[:, K + col : K + col + 1],
                        op0=eq,
                        op1=mul,
                    )
                    nc.tensor.matmul(
                        out=xa_ps[:],
                        lhsT=g_t[:],
                        rhs=s_t[:],
                        start=(t == 0),
                        stop=(t == T_b - 1),
                    )
                # epilogue: H1^T = W1^T @ xa  (chunks of 125 hid), relu+b1,
                # h2 = W2^T @ relu(H1)
                xa_sb = epool.tile([P, P], f32, name="xa_sb")
                nc.vector.tensor_scalar_add(out=xa_sb[:], in0=xa_ps[:], scalar1=0.0)
                h2_ps = ps_h2.tile([1, P], f32, name="h2_ps")
                for c in range(NCH):
                    h1_ps = ps_h1.tile([HCH, P], f32, name="h1_ps")
                    nc.tensor.matmul(
                        out=h1_ps[:],
                        lhsT=wts_t[:, c * HCH : (c + 1) * HCH],
                        rhs=xa_sb[:],
                        start=True,
                        stop=True,
                    )
                    rh_sb = epool.tile([HCH, P], f32, name="rh_sb")
                    nc.vector.tensor_scalar(
                        out=rh_sb[:],
                        in0=h1_ps[:],
                        scalar1=wts_t[:HCH, D_HID + c : D_HID + c + 1],
                        scalar2=0.0,
                        op0=add,
                        op1=mx,
                    )
                    nc.tensor.matmul(
                        out=h2_ps[:],
                        lhsT=wts_t[:HCH, D_HID + NCH + c : D_HID + NCH + c + 1],
                        rhs=rh_sb[:],
                        start=(c == 0),
                        stop=(c == NCH - 1),
                    )
                nc.vector.tensor_scalar_add(
                    out=h2sb[:, b * P : (b + 1) * P], in0=h2_ps[:], scalar1=0.0
                )

            # ---------------- AllGather h2 across cores ----------------
            h2loc = dpool.tile([shard, 1], f32, name="h2loc")
            h2all = dpool.tile([pad_n, 1], f32, name="h2all", addr_space="Shared")
            nc.sync.dma_start(out=h2loc[:], in_=h2sb[:])
            nc.gpsimd.collective_compute(
                "AllGather",
                mybir.AluOpType.bypass,
                replica_groups=[list(range(N_CORES))],
                ins=[h2loc.opt()],
                outs=[h2all.opt()],
            )

            # ---------------- pass 2: out = A_n @ h2 + b2 ----------------
            for b in range(bpc):
                o_ps = ps_acc.tile([P, 1], f32, name="o_ps")
                for t in range(T_b):
                    col = b * T_b + t
                    gh_t = gpool.tile([P, 1], f32, name="gh_t")
                    nc.gpsimd.indirect_dma_start(
                        out=gh_t[:],
                        out_offset=None,
                        in_=h2all[:],
                        in_offset=bass.IndirectOffsetOnAxis(
                            ap=idx_t[:, col : col + 1], axis=0
                        ),
                    )
                    s_t = spool.tile([P, P], f32, name="s_t")
                    nc.vector.tensor_scalar(
                        out=s_t[:],
                        in0=iota_t,
                        scalar1=meta_t[:, col : col + 1],
                        scalar2=meta_t[:, K + col : K + col + 1],
                        op0=eq,
                        op1=mul,
                    )
                    nc.tensor.matmul(
                        out=o_ps[:],
                        lhsT=s_t[:],
                        rhs=gh_t[:],
                        start=(t == 0),
                        stop=(t == T_b - 1),
                    )
                o_sb = epool.tile([P, 1], f32, name="o_sb")
                nc.vector.tensor_scalar_add(
                    out=o_sb[:], in0=o_ps[:], scalar1=float(b2_val)
                )
                nc.sync.dma_start(out=out_d[b * P : (b + 1) * P, :], in_=o_sb[:])
    nc.compile()
    return nc


def _make_in_maps(x, idx, dstl, wv, W1, b1, W2):
    iota_np = np.tile(np.arange(P, dtype=np.float32), (P, 1))
    b1c = np.asarray(b1, dtype=np.float32).reshape(NCH, HCH).T
    w2c = np.asarray(W2, dtype=np.float32).reshape(NCH, HCH).T
    wts = np.zeros((P, D_HID + 2 * NCH), dtype=np.float32)
    wts[:, :D_HID] = np.asarray(W1, dtype=np.float32)
    wts[:HCH, D_HID : D_HID + NCH] = b1c
    wts[:HCH, D_HID + NCH :] = w2c
    x = np.ascontiguousarray(np.asarray(x, dtype=np.float32))
    return [
        {
            "x": x,
            "idx": np.ascontiguousarray(idx[m]),
            "meta": np.ascontiguousarray(
                np.concatenate([dstl[m], wv[m], iota_np], axis=1)
            ),
            "wts": wts,
        }
        for m in range(N_CORES)
    ]


def kernel(x, edge_index, W1, b1, W2, b2, _trace=False):
    from concourse.bass_utils import run_bass_kernel_spmd

    x = np.asarray(x)
    n_nodes = x.shape[0]
    idx, dstl, wv, bpc, T_b, K = _preprocess(edge_index, n_nodes)
    b2_val = float(np.asarray(b2, dtype=np.float32).reshape(-1)[0])
    nc = _build_program(n_nodes, K, T_b, bpc, b2_val)
    in_maps = _make_in_maps(x, idx, dstl, wv, W1, b1, W2)
    res = run_bass_kernel_spmd(nc, in_maps, list(range(N_CORES)), trace=_trace)
    full = np.concatenate(
        [np.asarray(res.results[m]["out"]).reshape(-1) for m in range(N_CORES)]
    )[:n_nodes]
    out = np.ascontiguousarray(full.reshape(n_nodes, 1).astype(np.float32))
    if _trace:
        return out, res
    return out


# revision 22
# speedup vs baseline: 1.0001x; 1.0001x over previous
"""2-layer GCN (PyG GCNConv semantics) on 8 Trainium2 NeuronCores.

Math: out = A_n @ relu(A_n @ x @ W1 + b1) @ W2 + b2, where A_n is the
self-loop-augmented, symmetrically normalized adjacency.

Key reordering: A_n @ (x @ W1) == (A_n @ x) @ W1, so we propagate the
128-dim x (not the 500-dim hidden) in layer 1.  Layer 2 propagates the
scalar h2 = relu(H1) @ W2.

Sharding: nodes are split into 8 contiguous shards of 6272 (49 blocks of
128 dsts).  Edges (incl. self-loops) are partitioned by dst shard.  Per
128-dst block, edges are packed into [128-lane x T_b-tile] arrays; a
weighted one-hot matrix S_w[e, d] = (dst_e == d) * w_e is built per tile
with one dual-op tensor_scalar, and TensorE matmuls against gathered
src rows perform the segment-sum in PSUM.  Layer-2 scalar features are
AllGathered so pass 2 reuses the identical block structure.
"""

import numpy as np

P = 128
D_FEAT = 128
D_HID = 500
HCH = 125  # hidden chunk (D_HID = HCH * NCH)
NCH = 4
N_CORES = 8


def _preprocess(edge_index, n_nodes):
    bpc = int(np.ceil(n_nodes / (N_CORES * P)))  # blocks per core
    shard = bpc * P
    src = np.asarray(edge_index[0], dtype=np.int64)
    dst = np.asarray(edge_index[1], dtype=np.int64)
    loops = np.arange(n_nodes, dtype=np.int64)
    s = np.concatenate([src, loops])
    d = np.concatenate([dst, loops])
    deg = np.bincount(d, minlength=n_nodes).astype(np.float32)
    dinv = (1.0 / np.sqrt(deg)).astype(np.float32)
    w = dinv[s] * dinv[d]

    gblk = d // P  # global 128-dst block id; (core, blk) = divmod(gblk, bpc)
    order = np.argsort(gblk, kind="stable")
    s, d, w, gblk = s[order], d[order], w[order], gblk[order]

    n_gblk = N_CORES * bpc
    counts = np.bincount(gblk, minlength=n_gblk)
    # per-block tile count = max over cores (SPMD needs a uniform program)
    tiles_cb = np.maximum(1, -(-counts.reshape(N_CORES, bpc) // P))
    T_list = tiles_cb.max(axis=0).astype(np.int64)
    tstart = np.zeros(bpc + 1, dtype=np.int64)
    np.cumsum(T_list, out=tstart[1:])
    K = int(tstart[-1])  # tiles per core

    idx = np.zeros((N_CORES, P, K), dtype=np.int32)
    dstl = np.full((N_CORES, P, K), -1.0, dtype=np.float32)
    wv = np.zeros((N_CORES, P, K), dtype=np.float32)

    starts = np.zeros(n_gblk + 1, dtype=np.int64)
    np.cumsum(counts, out=starts[1:])
    r = np.arange(len(d), dtype=np.int64) - starts[gblk]
    core_e = gblk // bpc
    b_e = gblk % bpc
    tile_e = tstart[b_e] + r // P
    lane_e = r % P
    idx[core_e, lane_e, tile_e] = s.astype(np.int32)
    dstl[core_e, lane_e, tile_e] = (d % P).astype(np.float32)
    wv[core_e, lane_e, tile_e] = w
    return idx, dstl, wv, bpc, T_list, K


def _build_program(n_nodes, K, T_list, bpc, b2_val):
    from concourse import bass, mybir
    from concourse.bacc import Bacc
    import concourse.tile as tile

    f32 = mybir.dt.float32
    i32 = mybir.dt.int32
    shard = bpc * P
    pad_n = N_CORES * shard

    nc = Bacc()
    # meta packs [dstl | wv | iota] so one DMA (one semaphore source)
    # covers every DVE constant; three separate loads overflowed the
    # tensor_scalar instruction's sync-wait slots at codegen.
    MW = 2 * K + P
    WW = D_HID + 2 * NCH  # [W1 | b1 cols | w2 cols]
    x_d = nc.dram_tensor("x", [n_nodes, D_FEAT], f32, kind="ExternalInput")
    idx_d = nc.dram_tensor("idx", [P, K], i32, kind="ExternalInput")
    meta_d = nc.dram_tensor("meta", [P, MW], f32, kind="ExternalInput")
    wts_d = nc.dram_tensor("wts", [P, WW], f32, kind="ExternalInput")
    out_d = nc.dram_tensor("out", [shard, 1], f32, kind="ExternalOutput")

    eq = mybir.AluOpType.is_equal
    mul = mybir.AluOpType.mult
    add = mybir.AluOpType.add
    mx = mybir.AluOpType.max

    with tile.TileContext(nc) as tc:
        with (
            tc.tile_pool(name="consts", bufs=1) as cpool,
            tc.tile_pool(name="gbuf", bufs=8) as gpool,
            tc.tile_pool(name="sbuf_s", bufs=4) as spool,
            tc.tile_pool(name="epi", bufs=2) as epool,
            tc.tile_pool(name="ps_acc", bufs=2, space="PSUM") as ps_acc,
            tc.tile_pool(name="ps_h1", bufs=2, space="PSUM") as ps_h1,
            tc.tile_pool(name="ps_h2", bufs=2, space="PSUM") as ps_h2,
            tc.tile_pool(name="dram", bufs=1, space="DRAM") as dpool,
        ):
            idx_t = cpool.tile([P, K], i32, name="idx_t")
            nc.sync.dma_start(out=idx_t[:], in_=idx_d[:])
            meta_t = cpool.tile([P, MW], f32, name="meta_t")
            nc.sync.dma_start(out=meta_t[:], in_=meta_d[:])
            wts_t = cpool.tile([P, WW], f32, name="wts_t")
            nc.sync.dma_start(out=wts_t[:], in_=wts_d[:])
            iota_t = meta_t[:, 2 * K : 2 * K + P]
            h2sb = cpool.tile([1, shard], f32, name="h2sb")

            # ---------------- pass 1: xa = (A_n @ x) per 128-dst block,
            # then h2 = relu(xa @ W1 + b1) @ W2 ----------------
            col0 = 0
            for b in range(bpc):
                T_b = int(T_list[b])
                xa_ps = ps_acc.tile([P, P], f32, name="xa_ps")
                for t in range(T_b):
                    col = col0 + t
                    g_t = gpool.tile([P, D_FEAT], f32, name="g_t")
                    nc.gpsimd.indirect_dma_start(
                        out=g_t[:],
                        out_offset=None,
                        in_=x_d[:],
                        in_offset=bass.IndirectOffsetOnAxis(
                            ap=idx_t[:, col : col + 1], axis=0
                        ),
                    )
                    s_t = spool.tile([P, P], f32, name="s_t")
                    nc.vector.tensor_scalar(
                        out=s_t[:],
                        in0=iota_t,
                        scalar1=meta_t[:, col : col + 1],
                        scalar2=meta_t[:, K + col : K + col + 1],
                        op0=eq,
                        op1=mul,
                    )
                    nc.tensor.matmul(
                        out=xa_ps[:],
                        lhsT=g_t[:],
                        rhs=s_t[:],
                        start=(t == 0),
                        stop=(t == T_b - 1),
                    )
                # epilogue: H1^T = W1^T @ xa  (chunks of 125 hid), relu+b1,
                # h2 = W2^T @ relu(H1)
                xa_sb = epool.tile([P, P], f32, name="xa_sb")
                nc.vector.tensor_scalar_add(out=xa_sb[:], in0=xa_ps[:], scalar1=0.0)
                h2_ps = ps_h2.tile([1, P], f32, name="h2_ps")
                for c in range(NCH):
                    h1_ps = ps_h1.tile([HCH, P], f32, name="h1_ps")
                    nc.tensor.matmul(
                        out=h1_ps[:],
                        lhsT=wts_t[:, c * HCH : (c + 1) * HCH],
                        rhs=xa_sb[:],
                        start=True,
                        stop=True,
                    )
                    rh_sb = epool.tile([HCH, P], f32, name="rh_sb")
                    nc.vector.tensor_scalar(
                        out=rh_sb[:],
                        in0=h1_ps[:],
                        scalar1=wts_t[:HCH, D_HID + c : D_HID + c + 1],
                        scalar2=0.0,
                        op0=add,
                        op1=mx,
                    )
                    nc.tensor.matmul(
                        out=h2_ps[:],
                        lhsT=wts_t[:HCH, D_HID + NCH + c : D_HID + NCH + c + 1],
                        rhs=rh_sb[:],
                        start=(c == 0),
                        stop=(c == NCH - 1),
                    )
                nc.vector.tensor_scalar_add(
                    out=h2sb[:, b * P : (b + 1) * P], in0=h2_ps[:], scalar1=0.0
                )
                col0 += T_b

            # ---------------- AllGather h2 across cores ----------------
            h2loc = dpool.tile([shard, 1], f32, name="h2loc")
            h2all = dpool.tile([pad_n, 1], f32, name="h2all", addr_space="Shared")
            nc.sync.dma_start(out=h2loc[:], in_=h2sb[:])
            nc.gpsimd.collective_compute(
                "AllGather",
                mybir.AluOpType.bypass,
                replica_groups=[list(range(N_CORES))],
                ins=[h2loc.opt()],
                outs=[h2all.opt()],
            )

            # ---------------- pass 2: out = A_n @ h2 + b2 ----------------
            col0 = 0
            for b in range(bpc):
                T_b = int(T_list[b])
                o_ps = ps_acc.tile([P, 1], f32, name="o_ps")
                for t in range(T_b):
                    col = col0 + t
                    gh_t = gpool.tile([P, 1], f32, name="gh_t")
                    nc.gpsimd.indirect_dma_start(
                        out=gh_t[:],
                        out_offset=None,
                        in_=h2all[:],
                        in_offset=bass.IndirectOffsetOnAxis(
                            ap=idx_t[:, col : col + 1], axis=0
                        ),
                    )
                    s_t = spool.tile([P, P], f32, name="s_t")
                    nc.vector.tensor_scalar(
                        out=s_t[:],
                        in0=iota_t,
                        scalar1=meta_t[:, col : col + 1],
                        scalar2=meta_t[:, K + col : K + col + 1],
                        op0=eq,
                        op1=mul,
                    )
                    nc.tensor.matmul(
                        out=o_ps[:],
                        lhsT=s_t[:],
                        rhs=gh_t[:],
                        start=(t == 0),
                        stop=(t == T_b - 1),
                    )
                o_sb = epool.tile([P, 1], f32, name="o_sb")
                nc.vector.tensor_scalar_add(
                    out=o_sb[:], in0=o_ps[:], scalar1=float(b2_val)
                )
                nc.sync.dma_start(out=out_d[b * P : (b + 1) * P, :], in_=o_sb[:])
                col0 += T_b
    nc.compile()
    return nc


def _make_in_maps(x, idx, dstl, wv, W1, b1, W2):
    iota_np = np.tile(np.arange(P, dtype=np.float32), (P, 1))
    b1c = np.asarray(b1, dtype=np.float32).reshape(NCH, HCH).T
    w2c = np.asarray(W2, dtype=np.float32).reshape(NCH, HCH).T
    wts = np.zeros((P, D_HID + 2 * NCH), dtype=np.float32)
    wts[:, :D_HID] = np.asarray(W1, dtype=np.float32)
    wts[:HCH, D_HID : D_HID + NCH] = b1c
    wts[:HCH, D_HID + NCH :] = w2c
    x = np.ascontiguousarray(np.asarray(x, dtype=np.float32))
    return [
        {
            "x": x,
            "idx": np.ascontiguousarray(idx[m]),
            "meta": np.ascontiguousarray(
                np.concatenate([dstl[m], wv[m], iota_np], axis=1)
            ),
            "wts": wts,
        }
        for m in range(N_CORES)
    ]


def kernel(x, edge_index, W1, b1, W2, b2, _trace=False):
    from concourse.bass_utils import run_bass_kernel_spmd

    x = np.asarray(x)
    n_nodes = x.shape[0]
    idx, dstl, wv, bpc, T_list, K = _preprocess(edge_index, n_nodes)
    b2_val = float(np.asarray(b2, dtype=np.float32).reshape(-1)[0])
    nc = _build_program(n_nodes, K, T_list, bpc, b2_val)
    in_maps = _make_in_maps(x, idx, dstl, wv, W1, b1, W2)
    res = run_bass_kernel_spmd(nc, in_maps, list(range(N_CORES)), trace=_trace)
    full = np.concatenate(
        [np.asarray(res.results[m]["out"]).reshape(-1) for m in range(N_CORES)]
    )[:n_nodes]
    out = np.ascontiguousarray(full.reshape(n_nodes, 1).astype(np.float32))
    if _trace:
        return out, res
    return out


# revision 23
# speedup vs baseline: 1.0027x; 1.0026x over previous
"""2-layer GCN (PyG GCNConv semantics) on 8 Trainium2 NeuronCores.

Math: out = A_n @ relu(A_n @ x @ W1 + b1) @ W2 + b2, where A_n is the
self-loop-augmented, symmetrically normalized adjacency.

Key reordering: A_n @ (x @ W1) == (A_n @ x) @ W1, so we propagate the
128-dim x (not the 500-dim hidden) in layer 1.  Layer 2 propagates the
scalar h2 = relu(H1) @ W2.

Sharding: nodes are split into 8 contiguous shards of 6272 (49 blocks of
128 dsts).  Edges (incl. self-loops) are partitioned by dst shard.  Per
128-dst block, edges are packed into [128-lane x T_b-tile] arrays; a
weighted one-hot matrix S_w[e, d] = (dst_e == d) * w_e is built per tile
with one dual-op tensor_scalar, and TensorE matmuls against gathered
src rows perform the segment-sum in PSUM.  Layer-2 scalar features are
AllGathered so pass 2 reuses the identical block structure.
"""

import numpy as np

P = 128
D_FEAT = 128
D_HID = 500
HCH = 125  # hidden chunk (D_HID = HCH * NCH)
NCH = 4
N_CORES = 8


def _preprocess(edge_index, n_nodes):
    bpc = int(np.ceil(n_nodes / (N_CORES * P)))  # blocks per core
    shard = bpc * P
    src = np.asarray(edge_index[0], dtype=np.int64)
    dst = np.asarray(edge_index[1], dtype=np.int64)
    loops = np.arange(n_nodes, dtype=np.int64)
    s = np.concatenate([src, loops])
    d = np.concatenate([dst, loops])
    deg = np.bincount(d, minlength=n_nodes).astype(np.float32)
    dinv = (1.0 / np.sqrt(deg)).astype(np.float32)
    w = dinv[s] * dinv[d]

    gblk = d // P  # global 128-dst block id; (core, blk) = divmod(gblk, bpc)
    order = np.argsort(gblk, kind="stable")
    s, d, w, gblk = s[order], d[order], w[order], gblk[order]

    n_gblk = N_CORES * bpc
    counts = np.bincount(gblk, minlength=n_gblk)
    # per-block tile count = max over cores (SPMD needs a uniform program)
    tiles_cb = np.maximum(1, -(-counts.reshape(N_CORES, bpc) // P))
    T_list = tiles_cb.max(axis=0).astype(np.int64)
    tstart = np.zeros(bpc + 1, dtype=np.int64)
    np.cumsum(T_list, out=tstart[1:])
    K = int(tstart[-1])  # tiles per core

    idx = np.zeros((N_CORES, P, K), dtype=np.int32)
    dstl = np.full((N_CORES, P, K), -1.0, dtype=np.float32)
    wv = np.zeros((N_CORES, P, K), dtype=np.float32)

    starts = np.zeros(n_gblk + 1, dtype=np.int64)
    np.cumsum(counts, out=starts[1:])
    r = np.arange(len(d), dtype=np.int64) - starts[gblk]
    core_e = gblk // bpc
    b_e = gblk % bpc
    tile_e = tstart[b_e] + r // P
    lane_e = r % P
    idx[core_e, lane_e, tile_e] = s.astype(np.int32)
    dstl[core_e, lane_e, tile_e] = (d % P).astype(np.float32)
    wv[core_e, lane_e, tile_e] = w
    return idx, dstl, wv, bpc, T_list, K


def _build_program(n_nodes, K, T_list, bpc, b2_val):
    from concourse import bass, mybir
    from concourse.bacc import Bacc
    import concourse.tile as tile

    f32 = mybir.dt.float32
    i32 = mybir.dt.int32
    shard = bpc * P
    pad_n = N_CORES * shard

    nc = Bacc()
    # meta packs [dstl | wv | iota] so one DMA (one semaphore source)
    # covers every DVE constant; three separate loads overflowed the
    # tensor_scalar instruction's sync-wait slots at codegen.
    MW = 2 * K + P
    WW = D_HID + 2 * NCH  # [W1 | b1 cols | w2 cols]
    x_d = nc.dram_tensor("x", [n_nodes, D_FEAT], f32, kind="ExternalInput")
    idx_d = nc.dram_tensor("idx", [P, K], i32, kind="ExternalInput")
    meta_d = nc.dram_tensor("meta", [P, MW], f32, kind="ExternalInput")
    wts_d = nc.dram_tensor("wts", [P, WW], f32, kind="ExternalInput")
    out_d = nc.dram_tensor("out", [shard, 1], f32, kind="ExternalOutput")

    eq = mybir.AluOpType.is_equal
    mul = mybir.AluOpType.mult
    add = mybir.AluOpType.add
    mx = mybir.AluOpType.max

    with tile.TileContext(nc) as tc:
        with (
            tc.tile_pool(name="consts", bufs=1) as cpool,
            tc.tile_pool(name="gbuf", bufs=24) as gpool,
            tc.tile_pool(name="sbuf_s", bufs=10) as spool,
            tc.tile_pool(name="epi", bufs=3) as epool,
            tc.tile_pool(name="ps_acc", bufs=2, space="PSUM") as ps_acc,
            tc.tile_pool(name="ps_h1", bufs=2, space="PSUM") as ps_h1,
            tc.tile_pool(name="ps_h2", bufs=2, space="PSUM") as ps_h2,
            tc.tile_pool(name="dram", bufs=1, space="DRAM") as dpool,
        ):
            idx_t = cpool.tile([P, K], i32, name="idx_t")
            nc.sync.dma_start(out=idx_t[:], in_=idx_d[:])
            meta_t = cpool.tile([P, MW], f32, name="meta_t")
            nc.sync.dma_start(out=meta_t[:], in_=meta_d[:])
            wts_t = cpool.tile([P, WW], f32, name="wts_t")
            nc.sync.dma_start(out=wts_t[:], in_=wts_d[:])
            iota_t = meta_t[:, 2 * K : 2 * K + P]
            h2sb = cpool.tile([1, shard], f32, name="h2sb")

            # ---------------- pass 1: xa = (A_n @ x) per 128-dst block,
            # then h2 = relu(xa @ W1 + b1) @ W2 ----------------
            col0 = 0
            for b in range(bpc):
                T_b = int(T_list[b])
                xa_ps = ps_acc.tile([P, P], f32, name="xa_ps")
                for t in range(T_b):
                    col = col0 + t
                    g_t = gpool.tile([P, D_FEAT], f32, name="g_t")
                    nc.gpsimd.indirect_dma_start(
                        out=g_t[:],
                        out_offset=None,
                        in_=x_d[:],
                        in_offset=bass.IndirectOffsetOnAxis(
                            ap=idx_t[:, col : col + 1], axis=0
                        ),
                    )
                    s_t = spool.tile([P, P], f32, name="s_t")
                    nc.vector.tensor_scalar(
                        out=s_t[:],
                        in0=iota_t,
                        scalar1=meta_t[:, col : col + 1],
                        scalar2=meta_t[:, K + col : K + col + 1],
                        op0=eq,
                        op1=mul,
                    )
                    nc.tensor.matmul(
                        out=xa_ps[:],
                        lhsT=g_t[:],
                        rhs=s_t[:],
                        start=(t == 0),
                        stop=(t == T_b - 1),
                    )
                # epilogue: H1^T = W1^T @ xa  (chunks of 125 hid), relu+b1,
                # h2 = W2^T @ relu(H1)
                xa_sb = epool.tile([P, P], f32, name="xa_sb")
                nc.vector.tensor_scalar_add(out=xa_sb[:], in0=xa_ps[:], scalar1=0.0)
                h2_ps = ps_h2.tile([1, P], f32, name="h2_ps")
                for c in range(NCH):
                    h1_ps = ps_h1.tile([HCH, P], f32, name="h1_ps")
                    nc.tensor.matmul(
                        out=h1_ps[:],
                        lhsT=wts_t[:, c * HCH : (c + 1) * HCH],
                        rhs=xa_sb[:],
                        start=True,
                        stop=True,
                    )
                    rh_sb = epool.tile([HCH, P], f32, name="rh_sb")
                    nc.vector.tensor_scalar(
                        out=rh_sb[:],
                        in0=h1_ps[:],
                        scalar1=wts_t[:HCH, D_HID + c : D_HID + c + 1],
                        scalar2=0.0,
                        op0=add,
                        op1=mx,
                    )
                    nc.tensor.matmul(
                        out=h2_ps[:],
                        lhsT=wts_t[:HCH, D_HID + NCH + c : D_HID + NCH + c + 1],
                        rhs=rh_sb[:],
                        start=(c == 0),
                        stop=(c == NCH - 1),
                    )
                nc.vector.tensor_scalar_add(
                    out=h2sb[:, b * P : (b + 1) * P], in0=h2_ps[:], scalar1=0.0
                )
                col0 += T_b

            # ---------------- AllGather h2 across cores ----------------
            h2loc = dpool.tile([shard, 1], f32, name="h2loc")
            h2all = dpool.tile([pad_n, 1], f32, name="h2all", addr_space="Shared")
            nc.sync.dma_start(out=h2loc[:], in_=h2sb[:])
            nc.gpsimd.collective_compute(
                "AllGather",
                mybir.AluOpType.bypass,
                replica_groups=[list(range(N_CORES))],
                ins=[h2loc.opt()],
                outs=[h2all.opt()],
            )

            # ---------------- pass 2: out = A_n @ h2 + b2 ----------------
            col0 = 0
            for b in range(bpc):
                T_b = int(T_list[b])
                o_ps = ps_acc.tile([P, 1], f32, name="o_ps")
                for t in range(T_b):
                    col = col0 + t
                    gh_t = gpool.tile([P, 1], f32, name="gh_t")
                    nc.gpsimd.indirect_dma_start(
                        out=gh_t[:],
                        out_offset=None,
                        in_=h2all[:],
                        in_offset=bass.IndirectOffsetOnAxis(
                            ap=idx_t[:, col : col + 1], axis=0
                        ),
                    )
                    s_t = spool.tile([P, P], f32, name="s_t")
                    nc.vector.tensor_scalar(
                        out=s_t[:],
                        in0=iota_t,
                        scalar1=meta_t[:, col : col + 1],
                        scalar2=meta_t[:, K + col : K + col + 1],
                        op0=eq,
                        op1=mul,
                    )
                    nc.tensor.matmul(
                        out=o_ps[:],
                        lhsT=s_t[:],
                        rhs=gh_t[:],
                        start=(t == 0),
                        stop=(t == T_b - 1),
                    )
                o_sb = epool.tile([P, 1], f32, name="o_sb")
                nc.vector.tensor_scalar_add(
                    out=o_sb[:], in0=o_ps[:], scalar1=float(b2_val)
                )
                nc.sync.dma_start(out=out_d[b * P : (b + 1) * P, :], in_=o_sb[:])
                col0 += T_b
    nc.compile()
    return nc


def _make_in_maps(x, idx, dstl, wv, W1, b1, W2):
    iota_np = np.tile(np.arange(P, dtype=np.float32), (P, 1))
    b1c = np.asarray(b1, dtype=np.float32).reshape(NCH, HCH).T
    w2c = np.asarray(W2, dtype=np.float32).reshape(NCH, HCH).T
    wts = np.zeros((P, D_HID + 2 * NCH), dtype=np.float32)
    wts[:, :D_HID] = np.asarray(W1, dtype=np.float32)
    wts[:HCH, D_HID : D_HID + NCH] = b1c
    wts[:HCH, D_HID + NCH :] = w2c
    x = np.ascontiguousarray(np.asarray(x, dtype=np.float32))
    return [
        {
            "x": x,
            "idx": np.ascontiguousarray(idx[m]),
            "meta": np.ascontiguousarray(
                np.concatenate([dstl[m], wv[m], iota_np], axis=1)
            ),
            "wts": wts,
        }
        for m in range(N_CORES)
    ]


def kernel(x, edge_index, W1, b1, W2, b2, _trace=False):
    from concourse.bass_utils import run_bass_kernel_spmd

    x = np.asarray(x)
    n_nodes = x.shape[0]
    idx, dstl, wv, bpc, T_list, K = _preprocess(edge_index, n_nodes)
    b2_val = float(np.asarray(b2, dtype=np.float32).reshape(-1)[0])
    nc = _build_program(n_nodes, K, T_list, bpc, b2_val)
    in_maps = _make_in_maps(x, idx, dstl, wv, W1, b1, W2)
    res = run_bass_kernel_spmd(nc, in_maps, list(range(N_CORES)), trace=_trace)
    full = np.concatenate(
        [np.asarray(res.results[m]["out"]).reshape(-1) for m in range(N_CORES)]
    )[:n_nodes]
    out = np.ascontiguousarray(full.reshape(n_nodes, 1).astype(np.float32))
    if _trace:
        return out, res
    return out
